# revision 34
# baseline (speedup 1.0000x reference)
"""Cross-attention (B=4, Sq=4096, Sk=1024, H=16, D=1024) on 8 TRN2 NeuronCores.

Sharding: tensor-parallel by heads. Core c owns heads (2c, 2c+1), i.e. columns
[128c, 128c+128) of Wq/Wk/Wv and rows [128c, 128c+128) of Wo.

v2 design notes (vs v1 baseline at ~585us):
  - The PE clock is HAM-gated (1.2 GHz cold / ~1.95-2.4 GHz warm); v1 ran at
    the cold streaming rate because the inner loop serialized on the exp
    activation and on late xt DMAs through the in-order tensor queue.
  - Inner loop now interleaves next-batch projection + prev-batch out-proj
    matmuls between the exp-dependent attention matmuls, so the PE always has
    independent work while ACT computes exp.
  - Softmax normalization moved AFTER the AllToAll: nout is sent unnormalized
    together with its per-query sums row (v_aug col 0 = ones), then one
    reciprocal + one broadcast DMA + one big DVE multiply per batch replaces
    the per-i5 reciprocal/broadcast/multiply chain.
  - Small DMAs batched into single multi-dim DMAs (xt group = 1x2MB, yt =
    1x1.5MB, rv gather = 2 DMAs, sends = 2 per i5-pair).
  - Queues: tensor=matmuls, scalar=exp only, vector=DVE only, sync=input
    loads + broadcasts, gpsimd=sends/stores/collectives.

Host prep: x,y,W* are pre-transposed/pre-chunked and cast to bf16 on the host;
all matmuls run bf16 with fp32 PSUM accumulation; output is fp32.
"""

import numpy as np
import ml_dtypes

import concourse.bass as bass
import concourse.mybir as mybir
from concourse import bacc, tile
from concourse import bass_utils

BF16 = mybir.dt.bfloat16
F32 = mybir.dt.float32

B = 4
SQ = 4096
SK = 1024
D = 1024
DC = 768
NCORES = 8
SQL = SQ // NCORES  # 512 output rows per batch per core
KC = D // 128       # 8 contraction chunks for q-proj / out-proj
FC = DC // 128      # 6 contraction chunks for k/v-proj
JC = SK // 128      # 8 key chunks
NI = SQ // 512      # 8 query blocks of 512 per batch
NG = SQ // 1024     # 4 xt groups per batch (2 i5 blocks each)

Exp = mybir.ActivationFunctionType.Exp
Alu = mybir.AluOpType


def build_nc():
    nc = bacc.Bacc(
        "TRN2",
        target_bir_lowering=False,
        debug=False,
        num_devices=NCORES,
    )

    xt = nc.dram_tensor("xt", [B, KC, 128, SQ], BF16, kind="ExternalInput")
    yt = nc.dram_tensor("yt", [B, FC, 128, SK], BF16, kind="ExternalInput")
    wq = nc.dram_tensor("wq", [KC, 128, 128], BF16, kind="ExternalInput")
    wk = nc.dram_tensor("wk", [FC, 128, 128], BF16, kind="ExternalInput")
    wv = nc.dram_tensor("wv", [FC, 128, 128], BF16, kind="ExternalInput")
    wo = nc.dram_tensor("wo", [KC, 128, D], BF16, kind="ExternalInput")
    bq = nc.dram_tensor("bq", [128, 1], F32, kind="ExternalInput")
    bk = nc.dram_tensor("bk", [128, 1], F32, kind="ExternalInput")
    bvb = nc.dram_tensor("bvb", [128, 128], F32, kind="ExternalInput")
    bob = nc.dram_tensor("bob", [128, D], F32, kind="ExternalInput")
    out = nc.dram_tensor("out", [B, SQL, D], F32, kind="ExternalOutput")

    # DRAM bounce buffers for the per-batch AllToAll. Per dest core:
    # rows 0:64 = head A vals, 64:128 = head B vals (already normalized).
    send = [
        nc.dram_tensor(f"a2a_send_{b}", [NCORES, 128, 512], BF16, kind="Internal")
        for b in range(B)
    ]
    recv = [
        nc.dram_tensor(f"a2a_recv_{b}", [NCORES, 128, 512], BF16, kind="Internal")
        for b in range(B)
    ]

    with tile.TileContext(nc) as tc:
        _program(nc, tc, xt, yt, wq, wk, wv, wo, bq, bk, bvb, bob, out, send, recv)
    nc.finalize()
    return nc


def _program(nc, tc, xt, yt, wq, wk, wv, wo, bq, bk, bvb, bob, out, send, recv):
    from contextlib import ExitStack

    with ExitStack() as ctx:
        const = ctx.enter_context(tc.tile_pool(name="const", bufs=1))
        ytp = ctx.enter_context(tc.tile_pool(name="ytp", bufs=2))
        xtp = ctx.enter_context(tc.tile_pool(name="xtp", bufs=5))
        qtp = ctx.enter_context(tc.tile_pool(name="qtp", bufs=2))
        ktp = ctx.enter_context(tc.tile_pool(name="ktp", bufs=2))
        vtp = ctx.enter_context(tc.tile_pool(name="vtp", bufs=16))
        ep = ctx.enter_context(tc.tile_pool(name="ep", bufs=3))
        attp = ctx.enter_context(tc.tile_pool(name="attp", bufs=4))
        attup = ctx.enter_context(tc.tile_pool(name="attup", bufs=8))
        recp = ctx.enter_context(tc.tile_pool(name="recp", bufs=4))
        recbp = ctx.enter_context(tc.tile_pool(name="recbp", bufs=4))
        bcp = ctx.enter_context(tc.tile_pool(name="bcp", bufs=8))
        rvp = ctx.enter_context(tc.tile_pool(name="rvp", bufs=4))
        outp = ctx.enter_context(tc.tile_pool(name="outp", bufs=2))
        rbp = ctx.enter_context(tc.tile_pool(name="rbp", bufs=4, space="DRAM"))
        # PSUM: scores 2x2 banks + nout 2x1 + proj/outproj 2x1 = 8 banks
        scp = ctx.enter_context(tc.tile_pool(name="scp", bufs=2, space="PSUM"))
        noutp = ctx.enter_context(tc.tile_pool(name="noutp", bufs=2, space="PSUM"))
        projp = ctx.enter_context(tc.tile_pool(name="projp", bufs=2, space="PSUM"))

        # ---- constants / weights resident in SBUF
        bq_sb = const.tile([128, 1], F32, tag="bq")
        nc.sync.dma_start(out=bq_sb[:, :], in_=bq[:, :])
        bk_sb = const.tile([128, 1], F32, tag="bk")
        nc.sync.dma_start(out=bk_sb[:, :], in_=bk[:, :])
        bvb_sb = const.tile([128, 128], F32, tag="bvb")
        nc.sync.dma_start(out=bvb_sb[:, :], in_=bvb[:, :])

        wq_sb = const.tile([128, KC * 128], BF16, tag="wq")
        nc.sync.dma_start(
            out=wq_sb[:, :].rearrange("p (k c) -> p k c", k=KC),
            in_=wq[:, :, :].rearrange("k p c -> p k c"),
        )
        wk_sb = const.tile([128, FC * 128], BF16, tag="wk")
        wv_sb = const.tile([128, FC * 128], BF16, tag="wv")

        def emit_wkv_load():
            nc.sync.dma_start(
                out=wk_sb[:, :].rearrange("p (f c) -> p f c", f=FC),
                in_=wk[:, :, :].rearrange("f p c -> p f c"),
            )
            nc.sync.dma_start(
                out=wv_sb[:, :].rearrange("p (f c) -> p f c", f=FC),
                in_=wv[:, :, :].rearrange("f p c -> p f c"),
            )
        wo_sb = const.tile([128, KC * D], BF16, tag="wo")
        bob_sb = const.tile([128, D], F32, tag="bob")

        def emit_wo_load():
            # wo + bob are not needed until the first out-proj (~1 batch in);
            # loaded after the first batch of data DMAs to keep startup lean
            nc.sync.dma_start(
                out=wo_sb[:, :].rearrange("p (k c) -> p k c", k=KC),
                in_=wo[:, :, :].rearrange("k p c -> p k c"),
            )
            nc.sync.dma_start(out=bob_sb[:, :], in_=bob[:, :])

        yt_d = {}
        kt_d = {}
        qt_d = {}
        xt_d = {}
        v_tiles = {}
        att_d = {}
        rvs_d = {}
        o_d = {}

        def emit_yt_load(pb):
            t = ytp.tile([128, FC * SK], BF16, name=f"yt_{pb}", tag="yt")
            nc.sync.dma_start(
                out=t[:, :].rearrange("p (f c) -> p f c", f=FC),
                in_=yt[pb, :, :, :].rearrange("f p c -> p f c"),
            )
            yt_d[pb] = t
            kt_d[pb] = ktp.tile([128, SK], BF16, name=f"kt_{pb}", tag="kt")
            qt_d[pb] = qtp.tile([128, SQ], BF16, name=f"qt_{pb}", tag="qt")

        def emit_xt_load(pb, i5):
            t = xtp.tile([128, KC * 512], BF16, name=f"xt_{pb}_{i5}", tag="xt")
            nc.sync.dma_start(
                out=t[:, :].rearrange("p (k c) -> p k c", k=KC),
                in_=xt[pb, :, :, i5 * 512:(i5 + 1) * 512].rearrange("k p c -> p k c"),
            )
            xt_d[(pb, i5)] = t

        def emit_xt_slot(slot):
            # absolute q-block slot -> (batch, i5)
            if slot < B * NI:
                emit_xt_load(slot // NI, slot % NI)

        def emit_k_chain(pb, j2):
            yt_sb = yt_d[pb]
            kps = projp.tile([128, 512], F32, name=f"kps_{pb}_{j2}", tag="proj")
            for fc in range(FC):
                nc.tensor.matmul(
                    kps[:, :],
                    lhsT=wk_sb[:, fc * 128:(fc + 1) * 128],
                    rhs=yt_sb[:, fc * SK + j2 * 512: fc * SK + (j2 + 1) * 512],
                    start=(fc == 0),
                    stop=(fc == FC - 1),
                )
            nc.vector.tensor_scalar_add(
                kt_d[pb][:, j2 * 512:(j2 + 1) * 512], kps[:, :], bk_sb[:, :]
            )

        def emit_v_chain(pb, jc):
            # v_aug layout per tile [128, 130]:
            #   cols 0:64  = head-A values, col 64  = ones (A sums)
            #   cols 65:129 = head-B values, col 129 = ones (B sums)
            yt_sb = yt_d[pb]
            vps = projp.tile([128, 128], F32, name=f"vps_{pb}_{jc}", tag="proj")
            for fc in range(FC):
                nc.tensor.matmul(
                    vps[:, :],
                    lhsT=yt_sb[:, fc * SK + jc * 128: fc * SK + (jc + 1) * 128],
                    rhs=wv_sb[:, fc * 128:(fc + 1) * 128],
                    start=(fc == 0),
                    stop=(fc == FC - 1),
                )
            v_t = vtp.tile([128, 130], BF16, name=f"v_{pb}_{jc}", tag="vt")
            nc.vector.tensor_tensor(
                out=v_t[:, 0:130].rearrange("p (h x) -> p h x", h=2)[:, :, 0:64],
                in0=vps[:, :].rearrange("p (h x) -> p h x", h=2),
                in1=bvb_sb[:, :].rearrange("p (h x) -> p h x", h=2),
                op=Alu.add,
            )
            nc.vector.memset(v_t[:, 64:65], 1.0)
            nc.vector.memset(v_t[:, 129:130], 1.0)
            v_tiles[(pb, jc)] = v_t

        def emit_q_chain(pb, i5):
            xt_sb = xt_d.pop((pb, i5))
            qps = projp.tile([128, 512], F32, name=f"qps_{pb}_{i5}", tag="proj")
            for kc in range(KC):
                nc.tensor.matmul(
                    qps[:, :],
                    lhsT=wq_sb[:, kc * 128:(kc + 1) * 128],
                    rhs=xt_sb[:, kc * 512:(kc + 1) * 512],
                    start=(kc == 0),
                    stop=(kc == KC - 1),
                )
            nc.vector.tensor_scalar(
                out=qt_d[pb][:, i5 * 512:(i5 + 1) * 512],
                in0=qps[:, :],
                scalar1=bq_sb[:, :],
                scalar2=0.125,
                op0=Alu.add,
                op1=Alu.mult,
            )

        def emit_rv_gather(ob):
            # A2A(ob) done: one DMA gathers the normalized attention output
            # into out-proj lhsT layout [128 dvals, 8 src x 512 queries]
            rv_all = rvp.tile([128, KC * 512], BF16, name=f"rv_{ob}", tag="rv")
            nc.sync.dma_start(
                out=rv_all[:, :].rearrange("p (k c) -> p k c", k=KC),
                in_=recv[ob][:, :, :].rearrange("k p c -> p k c"),
            )
            rvs_d[ob] = rv_all

        def emit_outproj_chunk(ob, chunk):
            i1, eh = divmod(chunk, 2)
            rvs = rvs_d[ob]
            ops = projp.tile([128, 512], F32, name=f"ops_{ob}_{chunk}", tag="proj")
            for cc in range(KC):
                nc.tensor.matmul(
                    ops[:, :],
                    lhsT=rvs[:, cc * 512 + i1 * 128: cc * 512 + (i1 + 1) * 128],
                    rhs=wo_sb[:, cc * D + eh * 512: cc * D + (eh + 1) * 512],
                    start=(cc == 0),
                    stop=(cc == KC - 1),
                )
            if eh == 0:
                o_d[(ob, i1)] = outp.tile(
                    [128, 1024], F32, name=f"o_{ob}_{i1}", tag="o"
                )
            o_t = o_d[(ob, i1)]
            nc.vector.tensor_add(
                o_t[:, eh * 512:(eh + 1) * 512], ops[:, :],
                bob_sb[:, eh * 512:(eh + 1) * 512],
            )
            if eh == 1:
                nc.gpsimd.dma_start(
                    out=out[ob, i1 * 128:(i1 + 1) * 128, :], in_=o_t[:, :]
                )

        # ---- startup: batch 0 k/v projections + first q blocks. The rest of
        # batch 0's q-projections become fillers inside its attention loop.
        emit_xt_slot(0)
        emit_yt_load(0)
        emit_xt_slot(1)
        emit_wkv_load()
        emit_xt_slot(2)
        emit_q_chain(0, 0)
        emit_xt_slot(3)
        emit_q_chain(0, 1)
        for j2 in range(SK // 512):
            emit_k_chain(0, j2)
        for jc in range(JC):
            emit_v_chain(0, jc)
        for j in range(2, NI):
            emit_q_chain(0, j)
            if j + 2 < 10:
                emit_xt_slot(j + 2)
        emit_yt_load(1)
        emit_wo_load()

        # Out-proj chunk placement: A2A(0) completes only ~i5 6 of batch 1,
        # so chunks(0) start at (1,7) and spill into batch 2. Batches 1 and
        # 2's remaining chunks are reserved for the drain: ~34us of PE work
        # covering A2A(3)'s trigger latency + wire time so the PE stays warm
        # into the final out-projection. Batch 3 is ACT-bound either way.
        GATHER_I5 = {1: 6, 2: 3, 3: 2}
        pend_norm = []

        for b in range(B):
            kt_sb = kt_d[b]
            qt_sb = qt_d[b]

            for i5 in range(NI):
                # ---- filler units for this i5: independent PE work emitted
                # between the exp-dependent attention matmuls
                fill = []
                # xt prefetch: slot s is consumed at filler-time s-8, so
                # loading s=T+10 at time T gives a two-slot (~20us) lead
                fill.append(lambda s=b * NI + i5 + 10: emit_xt_slot(s))
                if b + 1 < B:
                    if i5 == 7 and b + 2 < B:
                        fill.append(lambda pb=b + 2: emit_yt_load(pb))
                    fill.append(lambda j=i5: emit_q_chain(b + 1, j))
                    if i5 < 2:
                        fill.append(lambda j=i5: emit_k_chain(b + 1, j))
                    fill.append(lambda j=i5: emit_v_chain(b + 1, j))
                if b > 0 and i5 == GATHER_I5[b]:
                    fill.append(lambda ob=b - 1: emit_rv_gather(ob))

                isl = slice(i5 * 512, (i5 + 1) * 512)
                na = noutp.tile([65, 512], F32, name=f"na_{b}_{i5}", tag="nout")
                nb = noutp.tile([65, 512], F32, name=f"nb_{b}_{i5}", tag="nout")

                def emit_scores(jc):
                    sc = scp.tile([128, 1024], F32, name=f"sc_{b}_{i5}_{jc}", tag="sc")
                    jsl = slice(jc * 128, (jc + 1) * 128)
                    # scoresT for both heads, row-tiled (K=64 each, concurrent)
                    nc.tensor.matmul(
                        sc[:, 0:512],
                        lhsT=kt_sb[0:64, jsl],
                        rhs=qt_sb[0:64, isl],
                        start=True, stop=True,
                    )
                    nc.tensor.matmul(
                        sc[:, 512:1024],
                        lhsT=kt_sb[64:128, jsl],
                        rhs=qt_sb[64:128, isl],
                        start=True, stop=True,
                    )
                    e_t = ep.tile([128, 1024], BF16, name=f"e_{b}_{i5}_{jc}", tag="e")
                    nc.scalar.activation(e_t[:, :], sc[:, :], Exp)
                    return e_t

                # software-pipelined over jc: scores(jc+1) and filler work run
                # while ACT computes exp(jc)
                e_cur = emit_scores(0)
                nfill = len(fill)
                for jc in range(JC):
                    # spread filler units across the jc slots
                    f0 = jc * nfill // JC
                    f1 = (jc + 1) * nfill // JC
                    for f in fill[f0:f1]:
                        f()
                    e_next = emit_scores(jc + 1) if jc + 1 < JC else None
                    v_t = v_tiles[(b, jc)]
                    nc.tensor.matmul(
                        na[:, :],
                        lhsT=v_t[:, 0:65],
                        rhs=e_cur[:, 0:512],
                        start=(jc == 0),
                        stop=(jc == JC - 1),
                    )
                    nc.tensor.matmul(
                        nb[:, :],
                        lhsT=v_t[:, 65:130],
                        rhs=e_cur[:, 512:1024],
                        start=(jc == 0),
                        stop=(jc == JC - 1),
                    )
                    e_cur = e_next

                # evacuate nout psum, normalize by the sums row (row 0), and
                # stage bf16 att tiles; one send DMA per (i5-pair, head)
                if i5 % 2 == 0:
                    att_d[0] = attp.tile([64, 1024], BF16, name=f"attA_{b}_{i5}", tag="att")
                    att_d[1] = attp.tile([64, 1024], BF16, name=f"attB_{b}_{i5}", tag="att")
                hsl = slice((i5 % 2) * 512, (i5 % 2) * 512 + 512)
                # emit the PREVIOUS i5's deferred broadcast+multiply first:
                # by now its rb ride has landed, so the gpsimd queue never
                # stalls on the DRAM round-trip
                for fn in pend_norm:
                    fn()
                pend_norm = []
                for h, nres in ((0, na), (1, nb)):
                    # psum is freed by two fast vector reads: a bf16 cast-copy
                    # (staging for the late gpsimd multiply) and the recip
                    # (reads psum directly; row 64 = sums, full tile because
                    # DVE ops need base_partition 0)
                    att_u = attup.tile([65, 512], BF16, name=f"au_{b}_{i5}_{h}", tag="au")
                    nc.vector.tensor_copy(att_u[:, :], nres[:, :])
                    rec = recp.tile([65, 512], F32, name=f"rec_{b}_{i5}_{h}", tag="rec")
                    nc.vector.reciprocal_approx_fast(out=rec[:, :], in_=nres[:, :])
                    recb = recbp.tile([65, 512], BF16, name=f"rcb_{b}_{i5}_{h}", tag="rcb")
                    nc.vector.tensor_copy(recb[:, :], rec[:, :])
                    rb = rbp.tile([1, 512], BF16, name=f"rb_{b}_{i5}_{h}", tag="rb")
                    nc.gpsimd.dma_start(out=rb[:, :], in_=recb[64:65, :])

                    def norm_tail(h=h, rb=rb, att_u=att_u, att=att_d[h],
                                  hsl=hsl, b=b, i5=i5):
                        bc = bcp.tile([64, 512], BF16, name=f"bc_{b}_{i5}_{h}", tag="bc")
                        nc.gpsimd.dma_start(
                            out=bc[:, :], in_=rb[0:1, :].to_broadcast([64, 512])
                        )
                        nc.gpsimd.tensor_mul(att[:, hsl], att_u[0:64, :], bc[:, :])
                        if i5 % 2 == 1:
                            nc.gpsimd.dma_start(
                                out=send[b][i5 - 1:i5 + 1, h * 64:(h + 1) * 64, :]
                                    .rearrange("d p c -> p d c"),
                                in_=att[:, :].rearrange("p (d c) -> p d c", d=2),
                            )
                    pend_norm.append(norm_tail)
                if i5 == NI - 1:
                    # batch boundary: flush immediately so the A2A can trigger
                    for fn in pend_norm:
                        fn()
                    pend_norm = []

            # ---- AllToAll for this batch: head-shard -> seq-shard
            nc.gpsimd.collective_compute(
                "AllToAll",
                Alu.bypass,
                replica_groups=[list(range(NCORES))],
                ins=[send[b][:, :, :].opt()],
                outs=[recv[b][:, :, :].opt()],
            )

        # ---- drain: ALL of batches 0-2's out-proj (~50us of PE work) covers
        # A2A(3)'s trigger latency + wire time so the PE stays warm, then
        # batch 3's out-projection runs at full clock
        for ob in range(3):
            for chunk in range(8):
                emit_outproj_chunk(ob, chunk)
        emit_rv_gather(B - 1)
        for chunk in range(8):
            emit_outproj_chunk(B - 1, chunk)


def prep_in_maps(x, y, Wq, bq, Wk, bk, Wv, bv, Wo, bo):
    bf = ml_dtypes.bfloat16
    x = np.asarray(x, np.float32)
    y = np.asarray(y, np.float32)
    xt = np.ascontiguousarray(x.transpose(0, 2, 1)).reshape(B, KC, 128, SQ).astype(bf)
    yt = np.ascontiguousarray(y.transpose(0, 2, 1)).reshape(B, FC, 128, SK).astype(bf)
    wo = np.ascontiguousarray(np.asarray(Wo, np.float32).reshape(KC, 128, D)).astype(bf)
    bob = np.ascontiguousarray(
        np.broadcast_to(np.asarray(bo, np.float32)[None, :], (128, D))
    )
    in_maps = []
    for c in range(NCORES):
        cs = slice(c * 128, (c + 1) * 128)
        in_maps.append({
            "xt": xt,
            "yt": yt,
            "wq": np.ascontiguousarray(np.asarray(Wq, np.float32)[:, cs].reshape(KC, 128, 128)).astype(bf),
            "wk": np.ascontiguousarray(np.asarray(Wk, np.float32)[:, cs].reshape(FC, 128, 128)).astype(bf),
            "wv": np.ascontiguousarray(np.asarray(Wv, np.float32)[:, cs].reshape(FC, 128, 128)).astype(bf),
            "wo": wo,
            "bq": np.ascontiguousarray(np.asarray(bq, np.float32)[cs].reshape(128, 1)),
            "bk": np.ascontiguousarray(np.asarray(bk, np.float32)[cs].reshape(128, 1)),
            "bvb": np.ascontiguousarray(
                np.broadcast_to(np.asarray(bv, np.float32)[cs][None, :], (128, 128))
            ),
            "bob": bob,
        })
    return in_maps


_NC_CACHE = None


def get_nc():
    global _NC_CACHE
    if _NC_CACHE is None:
        _NC_CACHE = build_nc()
    return _NC_CACHE


def run(in_maps, **kwargs):
    nc = get_nc()
    return bass_utils.run_bass_kernel_spmd(
        nc, in_maps, core_ids=list(range(NCORES)), **kwargs
    )


def gather(results):
    full = np.empty((B, SQ, D), np.float32)
    for c in range(NCORES):
        full[:, c * SQL:(c + 1) * SQL, :] = results[c]["out"]
    return full


def kernel(**inputs):
    in_maps = prep_in_maps(**inputs)
    res = run(in_maps)
    return gather(res.results)


if __name__ == "__main__":
    nc = build_nc()
    print("build OK")


# revision 36
# speedup vs baseline: 1.0881x; 1.0881x over previous
"""Cross-attention (B=4, Sq=4096, Sk=1024, H=16, D=1024) on 8 TRN2 NeuronCores.

Sharding: tensor-parallel by heads. Core c owns heads (2c, 2c+1), i.e. columns
[128c, 128c+128) of Wq/Wk/Wv and rows [128c, 128c+128) of Wo.

v2 design notes (vs v1 baseline at ~585us):
  - The PE clock is HAM-gated (1.2 GHz cold / ~1.95-2.4 GHz warm); v1 ran at
    the cold streaming rate because the inner loop serialized on the exp
    activation and on late xt DMAs through the in-order tensor queue.
  - Inner loop now interleaves next-batch projection + prev-batch out-proj
    matmuls between the exp-dependent attention matmuls, so the PE always has
    independent work while ACT computes exp.
  - Softmax normalization moved AFTER the AllToAll: nout is sent unnormalized
    together with its per-query sums row (v_aug col 0 = ones), then one
    reciprocal + one broadcast DMA + one big DVE multiply per batch replaces
    the per-i5 reciprocal/broadcast/multiply chain.
  - Small DMAs batched into single multi-dim DMAs (xt group = 1x2MB, yt =
    1x1.5MB, rv gather = 2 DMAs, sends = 2 per i5-pair).
  - Queues: tensor=matmuls, scalar=exp only, vector=DVE only, sync=input
    loads + broadcasts, gpsimd=sends/stores/collectives.

Host prep: x,y,W* are pre-transposed/pre-chunked and cast to bf16 on the host;
all matmuls run bf16 with fp32 PSUM accumulation; output is fp32.
"""

import numpy as np
import ml_dtypes

import concourse.bass as bass
import concourse.mybir as mybir
from concourse import bacc, tile
from concourse import bass_utils

BF16 = mybir.dt.bfloat16
F32 = mybir.dt.float32

B = 4
SQ = 4096
SK = 1024
D = 1024
DC = 768
NCORES = 8
SQL = SQ // NCORES  # 512 output rows per batch per core
KC = D // 128       # 8 contraction chunks for q-proj / out-proj
FC = DC // 128      # 6 contraction chunks for k/v-proj
JC = SK // 128      # 8 key chunks
NI = SQ // 512      # 8 query blocks of 512 per batch
NG = SQ // 1024     # 4 xt groups per batch (2 i5 blocks each)

Exp = mybir.ActivationFunctionType.Exp
Alu = mybir.AluOpType


def build_nc():
    nc = bacc.Bacc(
        "TRN2",
        target_bir_lowering=False,
        debug=False,
        num_devices=NCORES,
    )

    xt = nc.dram_tensor("xt", [B, KC, 128, SQ], BF16, kind="ExternalInput")
    yt = nc.dram_tensor("yt", [B, FC, 128, SK], BF16, kind="ExternalInput")
    wq = nc.dram_tensor("wq", [KC, 128, 128], BF16, kind="ExternalInput")
    wk = nc.dram_tensor("wk", [FC, 128, 128], BF16, kind="ExternalInput")
    wv = nc.dram_tensor("wv", [FC, 128, 128], BF16, kind="ExternalInput")
    wo = nc.dram_tensor("wo", [KC, 128, D], BF16, kind="ExternalInput")
    bq = nc.dram_tensor("bq", [128, 1], F32, kind="ExternalInput")
    bk = nc.dram_tensor("bk", [128, 1], F32, kind="ExternalInput")
    bvb = nc.dram_tensor("bvb", [128, 128], F32, kind="ExternalInput")
    bob = nc.dram_tensor("bob", [128, D], F32, kind="ExternalInput")
    out = nc.dram_tensor("out", [B, SQL, D], F32, kind="ExternalOutput")

    # DRAM bounce buffers for the per-batch AllToAll. Per dest core:
    # rows 0:64 = head A vals, 64:128 = head B vals (already normalized).
    send = [
        nc.dram_tensor(f"a2a_send_{b}", [NCORES, 128, 512], BF16, kind="Internal")
        for b in range(B)
    ]
    recv = [
        nc.dram_tensor(f"a2a_recv_{b}", [NCORES, 128, 512], BF16, kind="Internal")
        for b in range(B)
    ]

    with tile.TileContext(nc) as tc:
        _program(nc, tc, xt, yt, wq, wk, wv, wo, bq, bk, bvb, bob, out, send, recv)
    nc.finalize()
    return nc


def _program(nc, tc, xt, yt, wq, wk, wv, wo, bq, bk, bvb, bob, out, send, recv):
    from contextlib import ExitStack

    with ExitStack() as ctx:
        const = ctx.enter_context(tc.tile_pool(name="const", bufs=1))
        ytp = ctx.enter_context(tc.tile_pool(name="ytp", bufs=2))
        xtp = ctx.enter_context(tc.tile_pool(name="xtp", bufs=5))
        qtp = ctx.enter_context(tc.tile_pool(name="qtp", bufs=2))
        ktp = ctx.enter_context(tc.tile_pool(name="ktp", bufs=2))
        vtp = ctx.enter_context(tc.tile_pool(name="vtp", bufs=16))
        ep = ctx.enter_context(tc.tile_pool(name="ep", bufs=3))
        attp = ctx.enter_context(tc.tile_pool(name="attp", bufs=4))
        attup = ctx.enter_context(tc.tile_pool(name="attup", bufs=8))
        recp = ctx.enter_context(tc.tile_pool(name="recp", bufs=4))
        recbp = ctx.enter_context(tc.tile_pool(name="recbp", bufs=4))
        bcp = ctx.enter_context(tc.tile_pool(name="bcp", bufs=8))
        rvp = ctx.enter_context(tc.tile_pool(name="rvp", bufs=4))
        outp = ctx.enter_context(tc.tile_pool(name="outp", bufs=3))
        rbp = ctx.enter_context(tc.tile_pool(name="rbp", bufs=4, space="DRAM"))
        # PSUM: scores 2x2 banks + nout 2x1 + proj/outproj 2x1 = 8 banks
        scp = ctx.enter_context(tc.tile_pool(name="scp", bufs=2, space="PSUM"))
        noutp = ctx.enter_context(tc.tile_pool(name="noutp", bufs=2, space="PSUM"))
        projp = ctx.enter_context(tc.tile_pool(name="projp", bufs=2, space="PSUM"))

        # ---- constants / weights resident in SBUF
        bq_sb = const.tile([128, 1], F32, tag="bq")
        nc.sync.dma_start(out=bq_sb[:, :], in_=bq[:, :])
        bk_sb = const.tile([128, 1], F32, tag="bk")
        nc.sync.dma_start(out=bk_sb[:, :], in_=bk[:, :])
        bvb_sb = const.tile([128, 128], F32, tag="bvb")
        nc.sync.dma_start(out=bvb_sb[:, :], in_=bvb[:, :])

        wq_sb = const.tile([128, KC * 128], BF16, tag="wq")

        def emit_wq_load():
            nc.sync.dma_start(
                out=wq_sb[:, :].rearrange("p (k c) -> p k c", k=KC),
                in_=wq[:, :, :].rearrange("k p c -> p k c"),
            )
        wk_sb = const.tile([128, FC * 128], BF16, tag="wk")
        wv_sb = const.tile([128, FC * 128], BF16, tag="wv")

        def emit_wkv_load():
            nc.sync.dma_start(
                out=wk_sb[:, :].rearrange("p (f c) -> p f c", f=FC),
                in_=wk[:, :, :].rearrange("f p c -> p f c"),
            )
            nc.sync.dma_start(
                out=wv_sb[:, :].rearrange("p (f c) -> p f c", f=FC),
                in_=wv[:, :, :].rearrange("f p c -> p f c"),
            )
        wo_sb = const.tile([128, KC * D], BF16, tag="wo")
        bob_sb = const.tile([128, D], F32, tag="bob")

        def emit_wo_load():
            # wo + bob are not needed until the first out-proj (~1 batch in);
            # loaded after the first batch of data DMAs to keep startup lean
            nc.sync.dma_start(
                out=wo_sb[:, :].rearrange("p (k c) -> p k c", k=KC),
                in_=wo[:, :, :].rearrange("k p c -> p k c"),
            )
            nc.sync.dma_start(out=bob_sb[:, :], in_=bob[:, :])

        yt_d = {}
        kt_d = {}
        qt_d = {}
        xt_d = {}
        v_tiles = {}
        att_d = {}
        rvs_d = {}
        o_d = {}

        def emit_yt_load(pb):
            t = ytp.tile([128, FC * SK], BF16, name=f"yt_{pb}", tag="yt")
            nc.sync.dma_start(
                out=t[:, :].rearrange("p (f c) -> p f c", f=FC),
                in_=yt[pb, :, :, :].rearrange("f p c -> p f c"),
            )
            yt_d[pb] = t
            kt_d[pb] = ktp.tile([128, SK], BF16, name=f"kt_{pb}", tag="kt")
            qt_d[pb] = qtp.tile([128, SQ], BF16, name=f"qt_{pb}", tag="qt")

        def emit_xt_load(pb, i5):
            t = xtp.tile([128, KC * 512], BF16, name=f"xt_{pb}_{i5}", tag="xt")
            nc.sync.dma_start(
                out=t[:, :].rearrange("p (k c) -> p k c", k=KC),
                in_=xt[pb, :, :, i5 * 512:(i5 + 1) * 512].rearrange("k p c -> p k c"),
            )
            xt_d[(pb, i5)] = t

        def emit_xt_slot(slot):
            # absolute q-block slot -> (batch, i5)
            if slot < B * NI:
                emit_xt_load(slot // NI, slot % NI)

        def emit_k_chain(pb, j2):
            yt_sb = yt_d[pb]
            kps = projp.tile([128, 512], F32, name=f"kps_{pb}_{j2}", tag="proj")
            for fc in range(FC):
                nc.tensor.matmul(
                    kps[:, :],
                    lhsT=wk_sb[:, fc * 128:(fc + 1) * 128],
                    rhs=yt_sb[:, fc * SK + j2 * 512: fc * SK + (j2 + 1) * 512],
                    start=(fc == 0),
                    stop=(fc == FC - 1),
                )
            nc.vector.tensor_scalar_add(
                kt_d[pb][:, j2 * 512:(j2 + 1) * 512], kps[:, :], bk_sb[:, :]
            )

        def emit_v_chain(pb, jc):
            # v_aug layout per tile [128, 130]:
            #   cols 0:64  = head-A values, col 64  = ones (A sums)
            #   cols 65:129 = head-B values, col 129 = ones (B sums)
            yt_sb = yt_d[pb]
            vps = projp.tile([128, 128], F32, name=f"vps_{pb}_{jc}", tag="proj")
            for fc in range(FC):
                nc.tensor.matmul(
                    vps[:, :],
                    lhsT=yt_sb[:, fc * SK + jc * 128: fc * SK + (jc + 1) * 128],
                    rhs=wv_sb[:, fc * 128:(fc + 1) * 128],
                    start=(fc == 0),
                    stop=(fc == FC - 1),
                )
            v_t = vtp.tile([128, 130], BF16, name=f"v_{pb}_{jc}", tag="vt")
            nc.vector.tensor_tensor(
                out=v_t[:, 0:130].rearrange("p (h x) -> p h x", h=2)[:, :, 0:64],
                in0=vps[:, :].rearrange("p (h x) -> p h x", h=2),
                in1=bvb_sb[:, :].rearrange("p (h x) -> p h x", h=2),
                op=Alu.add,
            )
            nc.vector.memset(v_t[:, 64:65], 1.0)
            nc.vector.memset(v_t[:, 129:130], 1.0)
            v_tiles[(pb, jc)] = v_t

        def emit_q_chain(pb, i5):
            xt_sb = xt_d.pop((pb, i5))
            qps = projp.tile([128, 512], F32, name=f"qps_{pb}_{i5}", tag="proj")
            for kc in range(KC):
                nc.tensor.matmul(
                    qps[:, :],
                    lhsT=wq_sb[:, kc * 128:(kc + 1) * 128],
                    rhs=xt_sb[:, kc * 512:(kc + 1) * 512],
                    start=(kc == 0),
                    stop=(kc == KC - 1),
                )
            nc.vector.tensor_scalar(
                out=qt_d[pb][:, i5 * 512:(i5 + 1) * 512],
                in0=qps[:, :],
                scalar1=bq_sb[:, :],
                scalar2=0.125,
                op0=Alu.add,
                op1=Alu.mult,
            )

        def emit_rv_gather(ob):
            # A2A(ob) done: one DMA gathers the normalized attention output
            # into out-proj lhsT layout [128 dvals, 8 src x 512 queries]
            rv_all = rvp.tile([128, KC * 512], BF16, name=f"rv_{ob}", tag="rv")
            nc.sync.dma_start(
                out=rv_all[:, :].rearrange("p (k c) -> p k c", k=KC),
                in_=recv[ob][:, :, :].rearrange("k p c -> p k c"),
            )
            rvs_d[ob] = rv_all

        def emit_outproj_chunk(ob, chunk):
            i1, eh = divmod(chunk, 2)
            rvs = rvs_d[ob]
            ops = projp.tile([128, 512], F32, name=f"ops_{ob}_{chunk}", tag="proj")
            for cc in range(KC):
                nc.tensor.matmul(
                    ops[:, :],
                    lhsT=rvs[:, cc * 512 + i1 * 128: cc * 512 + (i1 + 1) * 128],
                    rhs=wo_sb[:, cc * D + eh * 512: cc * D + (eh + 1) * 512],
                    start=(cc == 0),
                    stop=(cc == KC - 1),
                )
            if eh == 0:
                o_d[(ob, i1)] = outp.tile(
                    [128, 1024], F32, name=f"o_{ob}_{i1}", tag="o"
                )
            o_t = o_d[(ob, i1)]
            nc.vector.tensor_add(
                o_t[:, eh * 512:(eh + 1) * 512], ops[:, :],
                bob_sb[:, eh * 512:(eh + 1) * 512],
            )
            if eh == 1:
                nc.sync.dma_start(
                    out=out[ob, i1 * 128:(i1 + 1) * 128, :], in_=o_t[:, :]
                )

        # ---- startup: batch 0 k/v projections + first q blocks. The rest of
        # batch 0's q-projections become fillers inside its attention loop.
        # startup ordered by DMA arrival: k/v inputs (wk+wv+yt ~1.9MB) land
        # first, so k/v-chains start the PE at ~6us; q path (wq+xt) follows.
        emit_wkv_load()
        emit_yt_load(0)
        emit_wq_load()
        emit_xt_slot(0)
        emit_xt_slot(1)
        for j2 in range(SK // 512):
            emit_k_chain(0, j2)
        for jc in range(JC):
            emit_v_chain(0, jc)
            if jc < 2:
                emit_xt_slot(2 + jc)
        for j in range(NI):
            emit_q_chain(0, j)
            if j + 4 < 10:
                emit_xt_slot(j + 4)
        emit_yt_load(1)
        emit_wo_load()

        # Out-proj chunk placement: A2A(0) completes only ~i5 6 of batch 1,
        # so chunks(0) start at (1,7) and spill into batch 2. Batches 1 and
        # 2's remaining chunks are reserved for the drain: ~34us of PE work
        # covering A2A(3)'s trigger latency + wire time so the PE stays warm
        # into the final out-projection. Batch 3 is ACT-bound either way.
        GATHER_I5 = {1: 6, 2: 3, 3: 2}
        OP_SCHED = {(3, i): [(0, i)] for i in range(8)}
        pend_norm = []

        for b in range(B):
            kt_sb = kt_d[b]
            qt_sb = qt_d[b]

            for i5 in range(NI):
                # ---- filler units for this i5: independent PE work emitted
                # between the exp-dependent attention matmuls
                fill = []
                # xt prefetch: slot s is consumed at filler-time s-8, so
                # loading s=T+10 at time T gives a two-slot (~20us) lead
                fill.append(lambda s=b * NI + i5 + 10: emit_xt_slot(s))
                if b + 1 < B:
                    if i5 == 7 and b + 2 < B:
                        fill.append(lambda pb=b + 2: emit_yt_load(pb))
                    fill.append(lambda j=i5: emit_q_chain(b + 1, j))
                    if i5 < 2:
                        fill.append(lambda j=i5: emit_k_chain(b + 1, j))
                    fill.append(lambda j=i5: emit_v_chain(b + 1, j))
                for ob, cc in OP_SCHED.get((b, i5), []):
                    fill.append(lambda ob=ob, cc=cc: emit_outproj_chunk(ob, cc))
                if b > 0 and i5 == GATHER_I5[b]:
                    fill.append(lambda ob=b - 1: emit_rv_gather(ob))

                isl = slice(i5 * 512, (i5 + 1) * 512)
                na = noutp.tile([65, 512], F32, name=f"na_{b}_{i5}", tag="nout")
                nb = noutp.tile([65, 512], F32, name=f"nb_{b}_{i5}", tag="nout")

                def emit_scores(jc):
                    sc = scp.tile([128, 1024], F32, name=f"sc_{b}_{i5}_{jc}", tag="sc")
                    jsl = slice(jc * 128, (jc + 1) * 128)
                    # scoresT for both heads, row-tiled (K=64 each, concurrent)
                    nc.tensor.matmul(
                        sc[:, 0:512],
                        lhsT=kt_sb[0:64, jsl],
                        rhs=qt_sb[0:64, isl],
                        start=True, stop=True,
                    )
                    nc.tensor.matmul(
                        sc[:, 512:1024],
                        lhsT=kt_sb[64:128, jsl],
                        rhs=qt_sb[64:128, isl],
                        start=True, stop=True,
                    )
                    e_t = ep.tile([128, 1024], BF16, name=f"e_{b}_{i5}_{jc}", tag="e")
                    nc.scalar.activation(e_t[:, :], sc[:, :], Exp)
                    return e_t

                # software-pipelined over jc: scores(jc+1) and filler work run
                # while ACT computes exp(jc)
                e_cur = emit_scores(0)
                nfill = len(fill)
                for jc in range(JC):
                    # spread filler units across the jc slots
                    f0 = jc * nfill // JC
                    f1 = (jc + 1) * nfill // JC
                    for f in fill[f0:f1]:
                        f()
                    e_next = emit_scores(jc + 1) if jc + 1 < JC else None
                    v_t = v_tiles[(b, jc)]
                    nc.tensor.matmul(
                        na[:, :],
                        lhsT=v_t[:, 0:65],
                        rhs=e_cur[:, 0:512],
                        start=(jc == 0),
                        stop=(jc == JC - 1),
                    )
                    nc.tensor.matmul(
                        nb[:, :],
                        lhsT=v_t[:, 65:130],
                        rhs=e_cur[:, 512:1024],
                        start=(jc == 0),
                        stop=(jc == JC - 1),
                    )
                    e_cur = e_next

                # evacuate nout psum, normalize by the sums row (row 0), and
                # stage bf16 att tiles; one send DMA per (i5-pair, head)
                if i5 % 2 == 0:
                    att_d[0] = attp.tile([64, 1024], BF16, name=f"attA_{b}_{i5}", tag="att")
                    att_d[1] = attp.tile([64, 1024], BF16, name=f"attB_{b}_{i5}", tag="att")
                hsl = slice((i5 % 2) * 512, (i5 % 2) * 512 + 512)
                # emit the PREVIOUS i5's deferred broadcast+multiply first:
                # by now its rb ride has landed, so the gpsimd queue never
                # stalls on the DRAM round-trip
                for fn in pend_norm:
                    fn()
                pend_norm = []
                for h, nres in ((0, na), (1, nb)):
                    # psum is freed by two fast vector reads: a bf16 cast-copy
                    # (staging for the late gpsimd multiply) and the recip
                    # (reads psum directly; row 64 = sums, full tile because
                    # DVE ops need base_partition 0)
                    att_u = attup.tile([65, 512], BF16, name=f"au_{b}_{i5}_{h}", tag="au")
                    nc.vector.tensor_copy(att_u[:, :], nres[:, :])
                    rec = recp.tile([65, 512], F32, name=f"rec_{b}_{i5}_{h}", tag="rec")
                    nc.vector.reciprocal_approx_fast(out=rec[:, :], in_=nres[:, :])
                    recb = recbp.tile([65, 512], BF16, name=f"rcb_{b}_{i5}_{h}", tag="rcb")
                    nc.vector.tensor_copy(recb[:, :], rec[:, :])
                    rb = rbp.tile([1, 512], BF16, name=f"rb_{b}_{i5}_{h}", tag="rb")
                    nc.sync.dma_start(out=rb[:, :], in_=recb[64:65, :])

                    def norm_tail(h=h, rb=rb, att_u=att_u, att=att_d[h],
                                  hsl=hsl, b=b, i5=i5):
                        bc = bcp.tile([64, 512], BF16, name=f"bc_{b}_{i5}_{h}", tag="bc")
                        nc.sync.dma_start(
                            out=bc[:, :], in_=rb[0:1, :].to_broadcast([64, 512])
                        )
                        nc.gpsimd.tensor_mul(att[:, hsl], att_u[0:64, :], bc[:, :])
                        if i5 % 2 == 1:
                            nc.sync.dma_start(
                                out=send[b][i5 - 1:i5 + 1, h * 64:(h + 1) * 64, :]
                                    .rearrange("d p c -> p d c"),
                                in_=att[:, :].rearrange("p (d c) -> p d c", d=2),
                            )
                    pend_norm.append(norm_tail)
                if i5 == NI - 1:
                    # batch boundary: flush immediately so the A2A can trigger
                    for fn in pend_norm:
                        fn()
                    pend_norm = []

            # ---- AllToAll for this batch: head-shard -> seq-shard
            nc.gpsimd.collective_compute(
                "AllToAll",
                Alu.bypass,
                replica_groups=[list(range(NCORES))],
                ins=[send[b][:, :, :].opt()],
                outs=[recv[b][:, :, :].opt()],
            )

        # ---- drain: ALL of batches 0-2's out-proj (~50us of PE work) covers
        # A2A(3)'s trigger latency + wire time so the PE stays warm, then
        # batch 3's out-projection runs at full clock
        for ob in (1, 2):
            for chunk in range(8):
                emit_outproj_chunk(ob, chunk)
        emit_rv_gather(B - 1)
        for chunk in range(8):
            emit_outproj_chunk(B - 1, chunk)


def prep_in_maps(x, y, Wq, bq, Wk, bk, Wv, bv, Wo, bo):
    bf = ml_dtypes.bfloat16
    x = np.asarray(x, np.float32)
    y = np.asarray(y, np.float32)
    xt = np.ascontiguousarray(x.transpose(0, 2, 1)).reshape(B, KC, 128, SQ).astype(bf)
    yt = np.ascontiguousarray(y.transpose(0, 2, 1)).reshape(B, FC, 128, SK).astype(bf)
    wo = np.ascontiguousarray(np.asarray(Wo, np.float32).reshape(KC, 128, D)).astype(bf)
    bob = np.ascontiguousarray(
        np.broadcast_to(np.asarray(bo, np.float32)[None, :], (128, D))
    )
    in_maps = []
    for c in range(NCORES):
        cs = slice(c * 128, (c + 1) * 128)
        in_maps.append({
            "xt": xt,
            "yt": yt,
            "wq": np.ascontiguousarray(np.asarray(Wq, np.float32)[:, cs].reshape(KC, 128, 128)).astype(bf),
            "wk": np.ascontiguousarray(np.asarray(Wk, np.float32)[:, cs].reshape(FC, 128, 128)).astype(bf),
            "wv": np.ascontiguousarray(np.asarray(Wv, np.float32)[:, cs].reshape(FC, 128, 128)).astype(bf),
            "wo": wo,
            "bq": np.ascontiguousarray(np.asarray(bq, np.float32)[cs].reshape(128, 1)),
            "bk": np.ascontiguousarray(np.asarray(bk, np.float32)[cs].reshape(128, 1)),
            "bvb": np.ascontiguousarray(
                np.broadcast_to(np.asarray(bv, np.float32)[cs][None, :], (128, 128))
            ),
            "bob": bob,
        })
    return in_maps


_NC_CACHE = None


def get_nc():
    global _NC_CACHE
    if _NC_CACHE is None:
        _NC_CACHE = build_nc()
    return _NC_CACHE


def run(in_maps, **kwargs):
    nc = get_nc()
    return bass_utils.run_bass_kernel_spmd(
        nc, in_maps, core_ids=list(range(NCORES)), **kwargs
    )


def gather(results):
    full = np.empty((B, SQ, D), np.float32)
    for c in range(NCORES):
        full[:, c * SQL:(c + 1) * SQL, :] = results[c]["out"]
    return full


def kernel(**inputs):
    in_maps = prep_in_maps(**inputs)
    res = run(in_maps)
    return gather(res.results)


if __name__ == "__main__":
    nc = build_nc()
    print("build OK")
